# revision 1
# baseline (speedup 1.0000x reference)
"""Causal multi-head attention (B=4, S=2048, D=1024, H=16) on 8 TRN2 cores.

Sharding: data-parallel over batch (4) x tensor-parallel over head groups (2).
Core c handles batch c//2, heads (c%2)*8 .. (c%2)*8+8.  Each core computes a
partial output y_c = attn_out_c @ Wo[rows of its heads]; the host sums the two
partials per batch and adds the bias.

Per-core kernel (matmuls in fp32r: full PE rate at free dim >= 256, inputs
rounded to 1-8-11; psum accumulation in fp32).  Projections and attention are
software-pipelined per 512-wide sequence chunk: chunk n's qT/kT/v projections
are emitted just before chunk n's attention, all pools coexist (no SBUF
reuse barriers), so exp/mask/normalize work overlaps the next chunk's
projection matmuls.

  projections: qT, kT = (x@Wq)^T, (x@Wk)^T in [adim, S] layout; v = x@Wv
           seq-major, augmented with a ones column per head (the ones column
           makes the attn@v matmul also emit the softmax denominators).
  attention (per si-chunk, head): scores^T tiles kT.T @ qT (causal: only
           kj <= diag tiles, diagonal tiles column-restricted), exp on ACT,
           diagonal masking on DVE over live columns, flash-style PSUM
           accumulation of [v|1].T @ exp, normalization via the PE-broadcast
           reciprocal of the sums row; then the output projection rows for
           the chunk (aoT.T @ Wo).
"""

import numpy as np

import concourse.bass as bass
import concourse.mybir as mybir
import concourse.tile as tile
from concourse.bacc import Bacc
from concourse.bass_utils import run_bass_kernel_spmd

F32 = mybir.dt.float32
F32R = mybir.dt.float32r
EXP = mybir.ActivationFunctionType.Exp

B, S, D = 4, 2048, 1024
H, DH = 16, 64
G = 2                # head groups (tensor-parallel factor)
HPC = H // G         # heads per core
AD = HPC * DH        # 512: per-core attention dim
P = 128
NK = D // P          # 8 contraction chunks for the projections
SI = 512             # si (query) chunk width
NCI = S // SI        # 4
NT = S // P          # 16 seq tiles
VW = DH + 1          # 65: v columns + ones column per head
GB = 2               # kj tiles per exp batch (sc psum = GB banks, x2 bufs)


def _emit(nc, tc, xt, wq, wk, wv, wo, masks, y):
    xt_r = xt.rearrange("(k p) (n s) -> n p k s", p=P, s=SI)
    with (
        tc.tile_pool(name="persist", bufs=1) as pp,
        tc.tile_pool(name="qpool", bufs=2) as qpool,
        tc.tile_pool(name="xpool", bufs=2) as xpool,
        tc.tile_pool(name="ao", bufs=2) as aop,
        tc.tile_pool(name="exp", bufs=4) as epool,
        tc.tile_pool(name="small", bufs=2) as spool,
        tc.tile_pool(name="yout", bufs=2) as yp,
        tc.tile_pool(name="ps_u", bufs=2, space="PSUM") as ps_u,
        tc.tile_pool(name="ps_sc", bufs=2, space="PSUM") as ps_sc,
        tc.tile_pool(name="ps_out", bufs=2, space="PSUM") as ps_out,
    ):
        # weight loads split per contraction chunk so the first projection
        # matmuls only wait for their own chunk's DMA, not the full 2MB
        wq_sb = pp.tile([P, NK, AD], F32R)
        wk_sb = pp.tile([P, NK, AD], F32R)
        wv_sb = pp.tile([P, NK, AD], F32R)
        wo_sb = pp.tile([P, AD // P, D], F32R)
        mask_sb = pp.tile([P, 2, SI], F32R)
        wq_r = wq.rearrange("(k p) m -> p k m", p=P)
        wk_r = wk.rearrange("(k p) m -> p k m", p=P)
        wv_r = wv.rearrange("(k p) m -> p k m", p=P)

        def emit_weight_loads():
            # spread the startup weight loads across three DGE issuers so the
            # first projection matmuls aren't serialized behind one queue
            for k in range(NK):
                nc.sync.dma_start(out=wq_sb[:, k, :], in_=wq_r[:, k, :])
            for k in range(NK):
                nc.sync.dma_start(out=wk_sb[:, k, :], in_=wk_r[:, k, :])
            for k in range(NK):
                nc.sync.dma_start(out=wv_sb[:, k, :], in_=wv_r[:, k, :])
            nc.sync.dma_start(out=mask_sb, in_=masks[:, :, :])
            nc.sync.dma_start(out=wo_sb, in_=wo.rearrange("(t p) m -> p t m", p=P))

        kts = [pp.tile([P, AD // P, SI], F32R, name=f"kt{n}") for n in range(NCI)]
        vts = [pp.tile([P, SI // P, HPC, VW], F32R, name=f"vt{n}") for n in range(NCI)]

        def emit_x_load(ci):
            xa = xpool.tile([P, NK // 2, SI], F32R, tag="xt", name="xa")
            for k in range(NK // 2):
                nc.sync.dma_start(out=xa[:, k, :], in_=xt_r[ci][:, k, :])
            xb = xpool.tile([P, NK // 2, SI], F32R, tag="xt", name="xb")
            for k in range(NK // 2):
                nc.sync.dma_start(out=xb[:, k, :], in_=xt_r[ci][:, NK // 2 + k, :])
            return xa, xb

        def emit_proj(ci, xab):
            # ---- projections for chunk n = ci ---------------------------
            xa, xb = xab

            def xk(k):
                return (xa if k < NK // 2 else xb)[:, k % (NK // 2), :]

            qt = qpool.tile([P, AD // P, SI], F32R, name="qt")
            for dst, w_sb, eng in ((qt, wq_sb, nc.vector), (kts[ci], wk_sb, nc.scalar)):
                for m in range(AD // P):
                    ps = ps_u.tile([P, SI], F32, tag="u", name="psp")
                    for k in range(NK):
                        nc.tensor.matmul(
                            ps,
                            w_sb[:, k, m * P : (m + 1) * P],
                            xk(k),
                            start=(k == 0),
                            stop=(k == NK - 1),
                        )
                    if eng is nc.vector:
                        eng.tensor_copy(dst[:, m, :], ps)
                    else:
                        eng.copy(dst[:, m, :], ps)
            for st in range(SI // P):
                ps = ps_u.tile([P, AD], F32, tag="u", name="psv")
                for k in range(NK):
                    nc.tensor.matmul(
                        ps,
                        xk(k)[:, st * P : (st + 1) * P],
                        wv_sb[:, k, :],
                        start=(k == 0),
                        stop=(k == NK - 1),
                    )
                nc.vector.tensor_copy(
                    vts[ci][:, st, :, 0:DH],
                    ps.rearrange("p (h d) -> p h d", d=DH),
                )
                nc.vector.memset(vts[ci][:, st, :, DH : DH + 1].bitcast(F32), 1.0)
            return qt

        def emit_attn(ci, qt, prev):
            # ---- attention for si chunk ci ------------------------------
            aoT = aop.tile([P, AD // P, SI], F32R, name="aoT")
            nkj = 4 * ci + 4
            for h in range(HPC):
                rb = (h % 2) * 64
                tq = h // 2
                outp = ps_out.tile([P, SI], F32, name="outp")
                for g0 in range(0, nkj, GB):
                    gsz = min(GB, nkj - g0)
                    scp = ps_sc.tile([P, GB, SI], F32, name="scp")
                    # diagonal tiles: columns < lo are fully masked; skip them
                    # in scores, exp, mask-mul and attn@v.  live condition
                    # f >= p + 128*jd == (f-lo) >= p + 128*mi.  The whole
                    # exp group uses the group's min lo so the ACT op only
                    # reads psum columns the score matmuls initialized.
                    los = []
                    for j in range(gsz):
                        jd = g0 + j - 4 * ci
                        los.append((min(jd, 2) * P if jd >= 0 else 0, jd))
                    g_lo = min(lo for lo, _ in los)
                    los = [(max(lo, g_lo), jd) for lo, jd in los]
                    for j in range(gsz):
                        kj = g0 + j
                        nc.tensor.matmul(
                            scp[:, j, g_lo:SI],
                            kts[kj // 4][rb : rb + 64, tq, (kj % 4) * P : (kj % 4 + 1) * P],
                            qt[rb : rb + 64, tq, g_lo:SI],
                            start=True,
                            stop=True,
                        )
                    ex = epool.tile([P, GB, SI], F32R, name="ex")
                    nc.scalar.activation(
                        ex[:, 0:gsz, g_lo:SI], scp[:, 0:gsz, g_lo:SI], EXP
                    )
                    for j in range(gsz):
                        kj = g0 + j
                        lo, jd = los[j]
                        if jd >= 0:
                            mi = jd - lo // P  # 0 or 1
                            nc.vector.tensor_mul(
                                ex[:, j, lo:SI],
                                ex[:, j, lo:SI],
                                mask_sb[:, mi, 0 : SI - lo],
                            )
                        nc.tensor.matmul(
                            outp[0:VW, lo:SI],
                            vts[kj // 4][:, kj % 4, h, :],
                            ex[:, j, lo:SI],
                            start=(kj == 0),
                            stop=(kj == nkj - 1),
                        )
                # epilogue: normalize by the sums row (row DH of outp).
                # 1/sums is replicated to 64 partitions via a K=1 PE outer
                # product with an all-ones row (mask1 is all-ones at f>=255).
                rcp = spool.tile([P, SI], F32R, tag="rcp", name="rcp")
                with nc.allow_low_precision("fp32r normalization"):
                    nc.vector.reciprocal(rcp[DH : DH + 1, :], outp[DH : DH + 1, :])
                rep_ps = ps_u.tile([64, SI], F32, tag="u", name="rep_ps")
                nc.tensor.matmul(
                    rep_ps,
                    mask_sb[DH : DH + 1, 1, SI - 64 : SI],
                    rcp[DH : DH + 1, :],
                    start=True,
                    stop=True,
                )
                rep = spool.tile([64, SI], F32, tag="rep", bufs=1, name="rep")
                nc.vector.tensor_copy(rep, rep_ps)
                dst = aoT[rb : rb + 64, tq, :]
                if rb == 0:
                    nc.vector.tensor_mul(dst, outp[0:DH, :], rep)
                else:
                    stg = spool.tile([P, SI], F32R, tag="rcp", name="stg")[0:64, :]
                    nc.vector.tensor_mul(stg, outp[0:DH, :], rep)
                    nc.sync.dma_start(out=dst, in_=stg)
                if prev is not None:
                    emit_p3_unit(ci - 1, prev, h // 2, h % 2)
            return aoT

        def emit_p3_unit(ao_ci, aoT, st, half):
            # one output-projection tile (st, half) for si chunk ao_ci
            ps = ps_u.tile([P, 512], F32, tag="u", name="ps3")
            for t in range(AD // P):
                nc.tensor.matmul(
                    ps,
                    aoT[:, t, st * P : (st + 1) * P],
                    wo_sb[:, t, half * 512 : (half + 1) * 512],
                    start=(t == 0),
                    stop=(t == AD // P - 1),
                )
            ysb = yp.tile([P, 512], F32, name="ysb")
            if half == 0:
                nc.scalar.copy(ysb, ps)
            else:
                nc.vector.tensor_copy(ysb, ps)
            nc.sync.dma_start(
                out=y[
                    ao_ci * SI + st * P : ao_ci * SI + (st + 1) * P,
                    half * 512 : (half + 1) * 512,
                ],
                in_=ysb,
            )

        # first chunk: interleave x and wq chunk loads so the first
        # projection group's dependencies complete earliest
        xa0 = xpool.tile([P, NK // 2, SI], F32R, tag="xt", name="xa")
        xb0 = xpool.tile([P, NK // 2, SI], F32R, tag="xt", name="xb")
        for k in range(NK // 2):
            nc.sync.dma_start(out=xa0[:, k, :], in_=xt_r[0][:, k, :])
            nc.sync.dma_start(out=wq_sb[:, k, :], in_=wq_r[:, k, :])
        for k in range(NK // 2):
            nc.sync.dma_start(out=xb0[:, k, :], in_=xt_r[0][:, NK // 2 + k, :])
            nc.sync.dma_start(out=wq_sb[:, NK // 2 + k, :], in_=wq_r[:, NK // 2 + k, :])
        x0 = (xa0, xb0)
        for k in range(NK):
            nc.sync.dma_start(out=wk_sb[:, k, :], in_=wk_r[:, k, :])
        for k in range(NK):
            nc.sync.dma_start(out=wv_sb[:, k, :], in_=wv_r[:, k, :])
        nc.sync.dma_start(out=mask_sb, in_=masks[:, :, :])
        nc.sync.dma_start(out=wo_sb, in_=wo.rearrange("(t p) m -> p t m", p=P))
        qts = {0: emit_proj(0, x0)}
        prev_ao = None
        for ci in range(NCI):
            if ci + 1 < NCI:
                qts[ci + 1] = emit_proj(ci + 1, emit_x_load(ci + 1))
            prev_ao = emit_attn(ci, qts.pop(ci), prev_ao)
        for st in range(SI // P):
            for half in range(2):
                emit_p3_unit(NCI - 1, prev_ao, st, half)


def build():
    nc = Bacc()
    xt = nc.dram_tensor("xt", [D, S], F32R, kind="ExternalInput")
    wq = nc.dram_tensor("wq", [D, AD], F32R, kind="ExternalInput")
    wk = nc.dram_tensor("wk", [D, AD], F32R, kind="ExternalInput")
    wv = nc.dram_tensor("wv", [D, AD], F32R, kind="ExternalInput")
    wo = nc.dram_tensor("wo", [AD, D], F32R, kind="ExternalInput")
    masks = nc.dram_tensor("masks", [P, 2, SI], F32R, kind="ExternalInput")
    y = nc.dram_tensor("y", [S, D], F32, kind="ExternalOutput")
    with tile.TileContext(nc) as tc:
        _emit(nc, tc, xt, wq, wk, wv, wo, masks, y)
    nc.compile()
    return nc


_NC = None


def _causal_masks():
    p = np.arange(P)[:, None]
    f = np.arange(SI)[None, :]
    return np.stack(
        [(f >= p).astype(np.float32), (f >= p + P).astype(np.float32)], axis=1
    )  # [P, 2, SI]


def run(x, Wq, Wk, Wv, Wo, bo, **run_kwargs):
    global _NC
    x = np.asarray(x, np.float32)
    Wq = np.asarray(Wq, np.float32)
    Wk = np.asarray(Wk, np.float32)
    Wv = np.asarray(Wv, np.float32)
    Wo = np.asarray(Wo, np.float32)
    bo = np.asarray(bo, np.float32)

    if _NC is None:
        _NC = build()

    masks = _causal_masks()
    wq_s = Wq * (1.0 / np.sqrt(DH))  # fold the 1/sqrt(dh) score scale into q
    in_maps = []
    for c in range(2 * B):
        b, g = divmod(c, G)
        cols = slice(g * AD, (g + 1) * AD)
        in_maps.append(
            {
                "xt": np.ascontiguousarray(x[b].T),
                "wq": np.ascontiguousarray(wq_s[:, cols]),
                "wk": np.ascontiguousarray(Wk[:, cols]),
                "wv": np.ascontiguousarray(Wv[:, cols]),
                "wo": np.ascontiguousarray(Wo[cols, :]),
                "masks": masks,
            }
        )

    res = run_bass_kernel_spmd(_NC, in_maps, core_ids=list(range(2 * B)), **run_kwargs)
    ys = [m["y"] for m in res.results]
    out = np.stack([ys[G * b] + ys[G * b + 1] for b in range(B)]) + bo
    return out.astype(np.float32), res


def kernel(**inputs):
    out, _ = run(**inputs)
    return out



# revision 9
# speedup vs baseline: 1.4549x; 1.4549x over previous
"""Causal multi-head attention (B=4, S=2048, D=1024, H=16) on 8 TRN2 cores.

Sharding: data-parallel over batch (4) x tensor-parallel over head groups (2).
Core c handles batch c//2, heads (c%2)*8 .. (c%2)*8+8.  Each core computes a
partial output y_c = attn_out_c @ Wo[rows of its heads]; the host sums the two
partials per batch and adds the bias.

v3: all matmul operands fp16 (psum accumulation fp32).  attn@v is transposed:
stationary = exp-scores block [keys, 128 queries], moving = v_aug [keys, 65]
-> out [queries, v|sum] costs 65 PE rows instead of 512 per 128-query tile.
Softmax denominators ride along as the ones-column; normalization is a
reciprocal + one stride-0-broadcast DVE multiply per head.  Normalized heads
assemble in [query, head*64] layout and PE-transpose back to [ad, query] for
the output projection.  Scheduling: head h's scores+exp are emitted before
head h-1's attn@v; the next chunk's projection matmul groups and deferred
output-projection units are interleaved into the head loop as PE filler so
the PE never idles while the ACT engine works through the exp chain.
"""

from collections import deque

import numpy as np

import concourse.bass as bass
import concourse.mybir as mybir
import concourse.tile as tile
from concourse.bacc import Bacc
from concourse.bass import AP
from concourse.bass_utils import run_bass_kernel_spmd

F32 = mybir.dt.float32
F16 = mybir.dt.float16
EXP = mybir.ActivationFunctionType.Exp

B, S, D = 4, 2048, 1024
H, DH = 16, 64
G = 2                # head groups (tensor-parallel factor)
HPC = H // G         # heads per core
AD = HPC * DH        # 512: per-core attention dim
P = 128
NK = D // P          # 8 contraction chunks for the projections
SI = 512             # si (query) chunk width
NCI = S // SI        # 4
VW = DH + 1          # 65: v columns + ones column per head
GB = 2               # kj tiles per exp batch (sc psum = GB banks, x2 bufs)

# scheduling constants (tuned against the TimelineSim cost model):
FILL = [0, 0, 1, 4]     # filler units popped per head in chunk ci
WARM = 6                # p-state warmup matmuls
PRE = 3                 # next-chunk heads whose off-diag scores+exp go early
FLUSH = 1               # filler pops right after each chunk's attn@v flush
KAO = 2                 # attn@v psum double-buffering
KEX = 40                # exp-tile ring depth
KF3 = "22334555"        # per-head filler pops in the last chunk
KPOPS = "1100"          # projection matmul groups popped per head, per chunk
KPREH = [5, 3, 2, 2]    # pre-computation window start head, per chunk


def _emit(nc, tc, xt, wq, wk, wv, wo, masks, ident, y):
    xt_r = xt.rearrange("(k p) (n s) -> n p k s", p=P, s=SI)
    with (
        tc.tile_pool(name="persist", bufs=1) as pp,
        tc.tile_pool(name="qpool", bufs=2) as qpool,
        tc.tile_pool(name="xpool", bufs=2) as xpool,
        tc.tile_pool(name="exp", bufs=KEX) as epool,
        tc.tile_pool(name="aoq", bufs=4) as aoqp,
        tc.tile_pool(name="small", bufs=4) as spool,
        tc.tile_pool(name="yout", bufs=6) as yp,
        tc.tile_pool(name="ps_u", bufs=2, space="PSUM") as ps_u,
        tc.tile_pool(name="ps_sc", bufs=2, space="PSUM") as ps_sc,
        tc.tile_pool(name="ps_ao", bufs=KAO, space="PSUM") as ps_ao,
    ):
        wq_sb = pp.tile([P, NK, AD], F16)
        wk_sb = pp.tile([P, NK, AD], F16)
        wv_sb = pp.tile([P, NK, AD], F16)
        wo_sb = pp.tile([P, AD // P, D], F16)
        mask_sb = pp.tile([P, 2, SI], F16)
        id_sb = pp.tile([P, P], F16)
        wq_r = wq.rearrange("(k p) m -> p k m", p=P)
        wk_r = wk.rearrange("(k p) m -> p k m", p=P)
        wv_r = wv.rearrange("(k p) m -> p k m", p=P)

        kts = [pp.tile([P, AD // P, SI], F16, name=f"kt{n}") for n in range(NCI)]
        vts = [pp.tile([P, SI // P, HPC, VW], F16, name=f"vt{n}") for n in range(NCI)]
        aoTs = [pp.tile([P, AD // P, SI], F16, name=f"aoT{n}") for n in range(NCI)]

        def emit_x_load(ci):
            xa = xpool.tile([P, NK // 2, SI], F16, tag="xt", name="xa")
            nc.sync.dma_start(out=xa, in_=xt_r[ci][:, 0 : NK // 2, :])
            xb = xpool.tile([P, NK // 2, SI], F16, tag="xt", name="xb")
            nc.sync.dma_start(out=xb, in_=xt_r[ci][:, NK // 2 : NK, :])
            return xa, xb

        def proj_groups(ci, xab):
            # Returns (qt, [closures]) - one closure per PE matmul group so
            # the caller can interleave them with attention work.
            xa, xb = xab

            def xk(k):
                return (xa if k < NK // 2 else xb)[:, k % (NK // 2), :]

            qt = qpool.tile([P, AD // P, SI], F16, name="qt")
            groups = []

            def qk_group(dst, w_sb, eng, m):
                ps = ps_u.tile([P, SI], F32, tag="u", name="psp")
                for k in range(NK):
                    nc.tensor.matmul(
                        ps,
                        w_sb[:, k, m * P : (m + 1) * P],
                        xk(k),
                        start=(k == 0),
                        stop=(k == NK - 1),
                    )
                if eng is nc.vector:
                    eng.tensor_copy(dst[:, m, :], ps)
                else:
                    eng.copy(dst[:, m, :], ps)

            def v_group(st):
                ps = ps_u.tile([P, AD], F32, tag="u", name="psv")
                for k in range(NK):
                    nc.tensor.matmul(
                        ps,
                        xk(k)[:, st * P : (st + 1) * P],
                        wv_sb[:, k, :],
                        start=(k == 0),
                        stop=(k == NK - 1),
                    )
                nc.vector.tensor_copy(
                    vts[ci][:, st, :, 0:DH],
                    ps.rearrange("p (h d) -> p h d", d=DH),
                )
                nc.vector.memset(vts[ci][:, st, :, DH : DH + 1], 1.0)

            for m in range(AD // P):
                groups.append(lambda m=m: qk_group(qt, wq_sb, nc.vector, m))
            for m in range(AD // P):
                groups.append(lambda m=m: qk_group(kts[ci], wk_sb, nc.vector, m))
            for st in range(SI // P):
                groups.append(lambda st=st: v_group(st))
            return qt, groups

        def emit_scores(ci, h, qt, gs, ge):
            # scores^T tiles (kT.T @ qT) + exp + causal masking for head h,
            # for kj-tile groups [gs, ge).  Off-diagonal groups (g < 2*ci)
            # only touch kts of earlier chunks, so they can be emitted one
            # chunk-phase early.
            rb = (h % 2) * 64
            tq = h // 2
            nkj = 4 * ci + 4
            exs = []
            for g0 in range(gs * GB, min(ge * GB, nkj), GB):
                gsz = min(GB, nkj - g0)
                # Each diagonal tile jd writes only its live columns
                # [jd*128, SI) - at fp16 any free size runs at full rate.  The
                # exp below still reads the group-min rectangle; the dead
                # columns hold stale-but-finite psum, are never consumed by
                # attn@v (qtile t only reads columns [t*128,(t+1)*128) with
                # t >= jd), and the mask row 0 shifted by jd*128 is exactly
                # the causal predicate f >= p + jd*128.
                los = []
                for j in range(gsz):
                    jd = g0 + j - 4 * ci
                    los.append((jd * P if jd >= 0 else 0, jd))
                g_lo = min(lo for lo, _ in los)
                scp = ps_sc.tile([P, GB, SI], F32, tag="sc", name="scp")
                for j in range(gsz):
                    kj = g0 + j
                    lo = los[j][0]
                    nc.tensor.matmul(
                        scp[:, j, lo:SI],
                        kts[kj // 4][rb : rb + 64, tq, (kj % 4) * P : (kj % 4 + 1) * P],
                        qt[rb : rb + 64, tq, lo:SI],
                        start=True,
                        stop=True,
                    )
                ex = epool.tile([P, GB, SI], F16, tag="ex", name="ex")
                nc.scalar.activation(ex[:, 0:gsz, g_lo:SI], scp[:, 0:gsz, g_lo:SI], EXP)
                for j in range(gsz):
                    lo, jd = los[j]
                    if jd >= 0:
                        nc.vector.tensor_mul(
                            ex[:, j, lo:SI],
                            ex[:, j, lo:SI],
                            mask_sb[:, 0, 0 : SI - lo],
                        )
                exs.append(ex)
            return exs

        def emit_attnv_norm(ci, h, exs, ao_q):
            # out[queries, v|sum] accumulation per 128-query tile, then
            # softmax-normalize via reciprocal + stride-0 broadcast multiply.
            ao_ps = ps_ao.tile([P, 4, P], F32, tag="ao", name="ao_ps")
            for t in range(4):
                last = 4 * ci + t
                for kj in range(last + 1):
                    nc.tensor.matmul(
                        ao_ps[:, t, 0:VW],
                        exs[kj // GB][:, kj % GB, t * P : (t + 1) * P],
                        vts[kj // 4][:, kj % 4, h, :],
                        start=(kj == 0),
                        stop=(kj == last),
                    )
            rc = spool.tile([P, 4, 1], F32, tag="rc", name="rc")
            nc.vector.reciprocal(rc, ao_ps[:, 0:4, DH : DH + 1])
            base = rc[:, 0:4, 0:1]
            bc = AP(base.tensor, base.offset, [list(base.ap[0]), list(base.ap[1]), [0, DH]])
            nc.vector.tensor_mul(ao_q[:, 0:4, h, :], ao_ps[:, 0:4, 0:DH], bc)

        def emit_transpose(t, ao_q, aoT, eng=None):
            # [query, ad] -> [ad, query] for one 128-query tile via PE
            # transpose-mode (4 128x128 blocks into one psum bank).
            pst = ps_u.tile([P, 4, 2 * P], F16, tag="u", name="pst")
            for c in range(AD // P):
                nc.tensor.transpose(
                    pst[:, c, 0:P], ao_q[:, t, 2 * c : 2 * c + 2, :], id_sb
                )
            if eng is nc.scalar:
                nc.scalar.copy(aoT[:, 0:4, t * P : (t + 1) * P], pst[:, 0:4, 0:P])
            else:
                nc.vector.tensor_copy(aoT[:, 0:4, t * P : (t + 1) * P], pst[:, 0:4, 0:P])

        def emit_outproj(ci, st, half, split=False, eng=None, pool=None):
            ps3 = (pool or ps_u).tile([P, 512], F32, tag="u" if pool is None else "sc", name="ps3")
            for c in range(AD // P):
                nc.tensor.matmul(
                    ps3,
                    aoTs[ci][:, c, st * P : (st + 1) * P],
                    wo_sb[:, c, half * 512 : (half + 1) * 512],
                    start=(c == 0),
                    stop=(c == AD // P - 1),
                )
            ysb = yp.tile([P, 512], F32, name="ysb")
            rows = y[ci * SI + st * P : ci * SI + (st + 1) * P, :]
            if split:
                # tail latency: copy the two halves on different engines and
                # overlap the two output DMAs
                nc.vector.tensor_copy(ysb[:, 0:256], ps3[:, 0:256])
                nc.scalar.copy(ysb[:, 256:512], ps3[:, 256:512])
                for q in range(2):
                    nc.sync.dma_start(
                        out=rows[:, half * 512 + q * 256 : half * 512 + (q + 1) * 256],
                        in_=ysb[:, q * 256 : (q + 1) * 256],
                    )
            elif eng is nc.scalar:
                nc.scalar.copy(ysb, ps3)
                nc.sync.dma_start(
                    out=rows[:, half * 512 : (half + 1) * 512],
                    in_=ysb,
                )
            else:
                nc.vector.tensor_copy(ysb, ps3)
                nc.sync.dma_start(
                    out=rows[:, half * 512 : (half + 1) * 512],
                    in_=ysb,
                )

        # ---- PE warmup: matmuls on scratch data issued before any DMA so
        # the p-state ramp completes during the startup DMA window --------
        if WARM:
            wsc = pp.tile([P, 256], F16)
            nc.vector.memset(wsc, 0.0)
            wps = ps_u.tile([P, 256], F32, tag="u", name="wps")
            for i in range(WARM):
                nc.tensor.matmul(wps, wsc[:, 0:P], wsc, start=(i == 0), stop=(i == WARM - 1))
        # ---- startup DMAs: interleave chunk-0 x with wq (k-pair batches)
        # so the first projection group's dependencies land earliest ------
        xa0 = xpool.tile([P, NK // 2, SI], F16, tag="xt", name="xa")
        xb0 = xpool.tile([P, NK // 2, SI], F16, tag="xt", name="xb")
        for k2 in range(NK // 4):
            nc.sync.dma_start(out=xa0[:, 2 * k2 : 2 * k2 + 2, :], in_=xt_r[0][:, 2 * k2 : 2 * k2 + 2, :])
            nc.sync.dma_start(out=wq_sb[:, 2 * k2 : 2 * k2 + 2, :], in_=wq_r[:, 2 * k2 : 2 * k2 + 2, :])
        for k2 in range(NK // 4):
            k = NK // 2 + 2 * k2
            nc.sync.dma_start(out=xb0[:, 2 * k2 : 2 * k2 + 2, :], in_=xt_r[0][:, k : k + 2, :])
            nc.sync.dma_start(out=wq_sb[:, k : k + 2, :], in_=wq_r[:, k : k + 2, :])
        nc.sync.dma_start(out=wk_sb[:, 0 : NK // 2, :], in_=wk_r[:, 0 : NK // 2, :])
        nc.sync.dma_start(out=wk_sb[:, NK // 2 : NK, :], in_=wk_r[:, NK // 2 : NK, :])
        nc.sync.dma_start(out=wv_sb[:, 0 : NK // 2, :], in_=wv_r[:, 0 : NK // 2, :])
        nc.sync.dma_start(out=wv_sb[:, NK // 2 : NK, :], in_=wv_r[:, NK // 2 : NK, :])
        nc.sync.dma_start(out=mask_sb, in_=masks[:, :, :])
        nc.sync.dma_start(out=id_sb, in_=ident[:, :])
        nc.sync.dma_start(out=wo_sb, in_=wo.rearrange("(t p) m -> p t m", p=P))

        qt0, groups0 = proj_groups(0, (xa0, xb0))
        for g in groups0:
            g()
        qts = {0: qt0}
        ao_qs = {}
        fillers = deque()
        pre_exs = {}
        for ci in range(NCI):
            ao_qs[ci] = aoqp.tile([P, 4, HPC, DH], F16, tag="aoq", name="ao_q")
            if ci + 1 < NCI:
                qts[ci + 1], pgroups = proj_groups(ci + 1, emit_x_load(ci + 1))
            else:
                pgroups = []
            pgroups = deque(pgroups)
            qt = qts.pop(ci)
            ngrp = 2 * ci + 2
            pend = None
            npopped = 0
            for h in range(HPC):
                if ci >= 1 and h == 0:
                    for t in range(4):
                        fillers.append(
                            lambda t=t, c=ci - 1: emit_transpose(t, ao_qs[c], aoTs[c])
                        )
                    for st in range(4):
                        for half in range(2):
                            fillers.append(
                                lambda st=st, half=half, c=ci - 1: emit_outproj(c, st, half)
                            )
                exs = pre_exs.pop((ci, h), None)
                if exs is None:
                    exs = emit_scores(ci, h, qt, 0, 2 * ci)
                exs = exs + emit_scores(ci, h, qt, 2 * ci, ngrp)
                if pend is not None:
                    emit_attnv_norm(ci, pend[0], pend[1], ao_qs[ci])
                pend = (h, exs)
                for _ in range(int(KPOPS[ci])):
                    if pgroups:
                        pgroups.popleft()()
                        npopped += 1
                if ci + 1 < NCI and ci + 1 >= 1 and KPREH[ci] <= h < KPREH[ci] + PRE:
                    hp = h - KPREH[ci]
                    # the pre-computed scores read qt(ci+1)[:, hp//2, :]; make
                    # sure that projection group has been emitted first
                    while npopped < hp // 2 + 1 and pgroups:
                        pgroups.popleft()()
                        npopped += 1
                    pre_exs[(ci + 1, hp)] = emit_scores(
                        ci + 1, hp, qts[ci + 1], 0, 2 * (ci + 1)
                    )
                npop = FILL[ci]
                if ci == NCI - 1 and len(KF3) == HPC:
                    npop = int(KF3[h])
                for _ in range(npop):
                    if fillers:
                        fillers.popleft()()
            while pgroups:
                pgroups.popleft()()
            emit_attnv_norm(ci, pend[0], pend[1], ao_qs[ci])
            for _ in range(FLUSH):
                if fillers:
                    fillers.popleft()()
        for t in range(4):
            emit_transpose(t, ao_qs[NCI - 1], aoTs[NCI - 1],
                           eng=(nc.scalar if t % 2 else nc.vector))
        while fillers:
            fillers.popleft()()
        for st in range(4):
            for half in range(2):
                emit_outproj(NCI - 1, st, half,
                             split=False,
                             eng=(nc.scalar if (2 * st + half) % 2 else nc.vector),
                             pool=(ps_sc if (2 * st + half) % 2 else None))


def build():
    nc = Bacc()
    xt = nc.dram_tensor("xt", [D, S], F16, kind="ExternalInput")
    wq = nc.dram_tensor("wq", [D, AD], F16, kind="ExternalInput")
    wk = nc.dram_tensor("wk", [D, AD], F16, kind="ExternalInput")
    wv = nc.dram_tensor("wv", [D, AD], F16, kind="ExternalInput")
    wo = nc.dram_tensor("wo", [AD, D], F16, kind="ExternalInput")
    masks = nc.dram_tensor("masks", [P, 2, SI], F16, kind="ExternalInput")
    ident = nc.dram_tensor("ident", [P, P], F16, kind="ExternalInput")
    y = nc.dram_tensor("y", [S, D], F32, kind="ExternalOutput")
    with tile.TileContext(nc) as tc:
        _emit(nc, tc, xt, wq, wk, wv, wo, masks, ident, y)
    nc.compile()
    return nc


_NC = None


def _causal_masks():
    p = np.arange(P)[:, None]
    f = np.arange(SI)[None, :]
    return np.stack(
        [(f >= p).astype(np.float32), (f >= p + P).astype(np.float32)], axis=1
    )  # [P, 2, SI]


def run(x, Wq, Wk, Wv, Wo, bo, **run_kwargs):
    global _NC
    x = np.asarray(x, np.float32)
    Wq = np.asarray(Wq, np.float32)
    Wk = np.asarray(Wk, np.float32)
    Wv = np.asarray(Wv, np.float32)
    Wo = np.asarray(Wo, np.float32)
    bo = np.asarray(bo, np.float32)

    if _NC is None:
        _NC = build()

    masks = _causal_masks().astype(np.float16)
    ident = np.eye(P, dtype=np.float16)
    wq_s = Wq * (1.0 / np.sqrt(DH))  # fold the 1/sqrt(dh) score scale into q
    in_maps = []
    for c in range(2 * B):
        b, g = divmod(c, G)
        cols = slice(g * AD, (g + 1) * AD)
        in_maps.append(
            {
                "xt": np.ascontiguousarray(x[b].T).astype(np.float16),
                "wq": np.ascontiguousarray(wq_s[:, cols]).astype(np.float16),
                "wk": np.ascontiguousarray(Wk[:, cols]).astype(np.float16),
                "wv": np.ascontiguousarray(Wv[:, cols]).astype(np.float16),
                "wo": np.ascontiguousarray(Wo[cols, :]).astype(np.float16),
                "masks": masks,
                "ident": ident,
            }
        )

    res = run_bass_kernel_spmd(_NC, in_maps, core_ids=list(range(2 * B)), **run_kwargs)
    ys = [m["y"] for m in res.results]
    out = np.stack([ys[G * b] + ys[G * b + 1] for b in range(B)]) + bo
    return out.astype(np.float32), res


def kernel(**inputs):
    out, _ = run(**inputs)
    return out


# revision 10
# speedup vs baseline: 1.4566x; 1.0012x over previous
"""Causal multi-head attention (B=4, S=2048, D=1024, H=16) on 8 TRN2 cores.

Sharding: data-parallel over batch (4) x tensor-parallel over head groups (2).
Core c handles batch c//2, heads (c%2)*8 .. (c%2)*8+8.  Each core computes a
partial output y_c = attn_out_c @ Wo[rows of its heads]; the host sums the two
partials per batch and adds the bias.

v3: all matmul operands fp16 (psum accumulation fp32).  attn@v is transposed:
stationary = exp-scores block [keys, 128 queries], moving = v_aug [keys, 65]
-> out [queries, v|sum] costs 65 PE rows instead of 512 per 128-query tile.
Softmax denominators ride along as the ones-column; normalization is a
reciprocal + one stride-0-broadcast DVE multiply per head.  Normalized heads
assemble in [query, head*64] layout and PE-transpose back to [ad, query] for
the output projection.  Scheduling: head h's scores+exp are emitted before
head h-1's attn@v; the next chunk's projection matmul groups and deferred
output-projection units are interleaved into the head loop as PE filler so
the PE never idles while the ACT engine works through the exp chain.
"""

from collections import deque

import numpy as np

import concourse.bass as bass
import concourse.mybir as mybir
import concourse.tile as tile
from concourse.bacc import Bacc
from concourse.bass import AP
from concourse.bass_utils import run_bass_kernel_spmd

F32 = mybir.dt.float32
F16 = mybir.dt.float16
EXP = mybir.ActivationFunctionType.Exp

B, S, D = 4, 2048, 1024
H, DH = 16, 64
G = 2                # head groups (tensor-parallel factor)
HPC = H // G         # heads per core
AD = HPC * DH        # 512: per-core attention dim
P = 128
NK = D // P          # 8 contraction chunks for the projections
SI = 512             # si (query) chunk width
NCI = S // SI        # 4
VW = DH + 1          # 65: v columns + ones column per head
GB = 2               # kj tiles per exp batch (sc psum = GB banks, x2 bufs)

# scheduling constants (tuned against the TimelineSim cost model):
FILL = [0, 0, 1, 4]     # filler units popped per head in chunk ci
WARM = 6                # p-state warmup matmuls
PRE = 3                 # next-chunk heads whose off-diag scores+exp go early
FLUSH = 1               # filler pops right after each chunk's attn@v flush
KAO = 2                 # attn@v psum double-buffering
KEX = 40                # exp-tile ring depth
KF3 = "22334555"        # per-head filler pops in the last chunk
KPOPS = "1100"          # projection matmul groups popped per head, per chunk
KPREH = [5, 3, 2, 2]    # pre-computation window start head, per chunk


def _emit(nc, tc, xt, wq, wk, wv, wo, masks, ident, y):
    xt_r = xt.rearrange("(k p) (n s) -> n p k s", p=P, s=SI)
    with (
        tc.tile_pool(name="persist", bufs=1) as pp,
        tc.tile_pool(name="qpool", bufs=2) as qpool,
        tc.tile_pool(name="xpool", bufs=2) as xpool,
        tc.tile_pool(name="exp", bufs=KEX) as epool,
        tc.tile_pool(name="aoq", bufs=4) as aoqp,
        tc.tile_pool(name="small", bufs=4) as spool,
        tc.tile_pool(name="yout", bufs=6) as yp,
        tc.tile_pool(name="ps_u", bufs=2, space="PSUM") as ps_u,
        tc.tile_pool(name="ps_sc", bufs=2, space="PSUM") as ps_sc,
        tc.tile_pool(name="ps_ao", bufs=KAO, space="PSUM") as ps_ao,
    ):
        wq_sb = pp.tile([P, NK, AD], F16)
        wk_sb = pp.tile([P, NK, AD], F16)
        wv_sb = pp.tile([P, NK, AD], F16)
        wo_sb = pp.tile([P, AD // P, D], F16)
        mask_sb = pp.tile([P, 2, SI], F16)
        id_sb = pp.tile([P, P], F16)
        wq_r = wq.rearrange("(k p) m -> p k m", p=P)
        wk_r = wk.rearrange("(k p) m -> p k m", p=P)
        wv_r = wv.rearrange("(k p) m -> p k m", p=P)

        kts = [pp.tile([P, AD // P, SI], F16, name=f"kt{n}") for n in range(NCI)]
        vts = [pp.tile([P, SI // P, HPC, VW], F16, name=f"vt{n}") for n in range(NCI)]
        aoTs = [pp.tile([P, AD // P, SI], F16, name=f"aoT{n}") for n in range(NCI)]

        def emit_x_load(ci):
            xa = xpool.tile([P, NK // 2, SI], F16, tag="xt", name="xa")
            nc.sync.dma_start(out=xa, in_=xt_r[ci][:, 0 : NK // 2, :])
            xb = xpool.tile([P, NK // 2, SI], F16, tag="xt", name="xb")
            nc.sync.dma_start(out=xb, in_=xt_r[ci][:, NK // 2 : NK, :])
            return xa, xb

        def proj_groups(ci, xab):
            # Returns (qt, [closures]) - one closure per PE matmul group so
            # the caller can interleave them with attention work.
            xa, xb = xab

            def xk(k):
                return (xa if k < NK // 2 else xb)[:, k % (NK // 2), :]

            qt = qpool.tile([P, AD // P, SI], F16, name="qt")
            groups = []

            def qk_group(dst, w_sb, eng, m):
                ps = ps_u.tile([P, SI], F32, tag="u", name="psp")
                for k in range(NK):
                    nc.tensor.matmul(
                        ps,
                        w_sb[:, k, m * P : (m + 1) * P],
                        xk(k),
                        start=(k == 0),
                        stop=(k == NK - 1),
                    )
                if eng is nc.vector:
                    eng.tensor_copy(dst[:, m, :], ps)
                else:
                    eng.copy(dst[:, m, :], ps)

            def v_group(st):
                ps = ps_u.tile([P, AD], F32, tag="u", name="psv")
                for k in range(NK):
                    nc.tensor.matmul(
                        ps,
                        xk(k)[:, st * P : (st + 1) * P],
                        wv_sb[:, k, :],
                        start=(k == 0),
                        stop=(k == NK - 1),
                    )
                nc.vector.tensor_copy(
                    vts[ci][:, st, :, 0:DH],
                    ps.rearrange("p (h d) -> p h d", d=DH),
                )
                nc.vector.memset(vts[ci][:, st, :, DH : DH + 1], 1.0)

            for m in range(AD // P):
                groups.append(lambda m=m: qk_group(qt, wq_sb, nc.vector, m))
            for m in range(AD // P):
                groups.append(lambda m=m: qk_group(kts[ci], wk_sb, nc.vector, m))
            for st in range(SI // P):
                groups.append(lambda st=st: v_group(st))
            return qt, groups

        def emit_scores(ci, h, qt, gs, ge):
            # scores^T tiles (kT.T @ qT) + exp + causal masking for head h,
            # for kj-tile groups [gs, ge).  Off-diagonal groups (g < 2*ci)
            # only touch kts of earlier chunks, so they can be emitted one
            # chunk-phase early.
            rb = (h % 2) * 64
            tq = h // 2
            nkj = 4 * ci + 4
            exs = []
            for g0 in range(gs * GB, min(ge * GB, nkj), GB):
                gsz = min(GB, nkj - g0)
                # Each diagonal tile jd writes only its live columns
                # [jd*128, SI) - at fp16 any free size runs at full rate.  The
                # exp below still reads the group-min rectangle; the dead
                # columns hold stale-but-finite psum, are never consumed by
                # attn@v (qtile t only reads columns [t*128,(t+1)*128) with
                # t >= jd), and the mask row 0 shifted by jd*128 is exactly
                # the causal predicate f >= p + jd*128.
                los = []
                for j in range(gsz):
                    jd = g0 + j - 4 * ci
                    los.append((jd * P if jd >= 0 else 0, jd))
                g_lo = min(lo for lo, _ in los)
                scp = ps_sc.tile([P, GB, SI], F32, tag="sc", name="scp")
                for j in range(gsz):
                    kj = g0 + j
                    lo = los[j][0]
                    nc.tensor.matmul(
                        scp[:, j, lo:SI],
                        kts[kj // 4][rb : rb + 64, tq, (kj % 4) * P : (kj % 4 + 1) * P],
                        qt[rb : rb + 64, tq, lo:SI],
                        start=True,
                        stop=True,
                    )
                ex = epool.tile([P, GB, SI], F16, tag="ex", name="ex")
                nc.scalar.activation(ex[:, 0:gsz, g_lo:SI], scp[:, 0:gsz, g_lo:SI], EXP)
                for j in range(gsz):
                    lo, jd = los[j]
                    if jd >= 0:
                        nc.vector.tensor_mul(
                            ex[:, j, lo:SI],
                            ex[:, j, lo:SI],
                            mask_sb[:, 0, 0 : SI - lo],
                        )
                exs.append(ex)
            return exs

        def emit_attnv_norm(ci, h, exs, ao_q):
            # out[queries, v|sum] accumulation per 128-query tile, then
            # softmax-normalize via reciprocal + stride-0 broadcast multiply.
            ao_ps = ps_ao.tile([P, 4, P], F32, tag="ao", name="ao_ps")
            for t in range(4):
                last = 4 * ci + t
                for kj in range(last + 1):
                    nc.tensor.matmul(
                        ao_ps[:, t, 0:VW],
                        exs[kj // GB][:, kj % GB, t * P : (t + 1) * P],
                        vts[kj // 4][:, kj % 4, h, :],
                        start=(kj == 0),
                        stop=(kj == last),
                    )
            rc = spool.tile([P, 4, 1], F32, tag="rc", name="rc")
            nc.vector.reciprocal(rc, ao_ps[:, 0:4, DH : DH + 1])
            base = rc[:, 0:4, 0:1]
            bc = AP(base.tensor, base.offset, [list(base.ap[0]), list(base.ap[1]), [0, DH]])
            nc.vector.tensor_mul(ao_q[:, 0:4, h, :], ao_ps[:, 0:4, 0:DH], bc)

        def emit_transpose(t, ao_q, aoT, eng=None):
            # [query, ad] -> [ad, query] for one 128-query tile via PE
            # transpose-mode (4 128x128 blocks into one psum bank).
            pst = ps_u.tile([P, 4, 2 * P], F16, tag="u", name="pst")
            for c in range(AD // P):
                nc.tensor.transpose(
                    pst[:, c, 0:P], ao_q[:, t, 2 * c : 2 * c + 2, :], id_sb
                )
            if eng is nc.scalar:
                nc.scalar.copy(aoT[:, 0:4, t * P : (t + 1) * P], pst[:, 0:4, 0:P])
            else:
                nc.vector.tensor_copy(aoT[:, 0:4, t * P : (t + 1) * P], pst[:, 0:4, 0:P])

        def emit_outproj(ci, st, half, split=False, eng=None, pool=None):
            ps3 = (pool or ps_u).tile([P, 512], F32, tag="u" if pool is None else "sc", name="ps3")
            for c in range(AD // P):
                nc.tensor.matmul(
                    ps3,
                    aoTs[ci][:, c, st * P : (st + 1) * P],
                    wo_sb[:, c, half * 512 : (half + 1) * 512],
                    start=(c == 0),
                    stop=(c == AD // P - 1),
                )
            ysb = yp.tile([P, 512], F32, name="ysb")
            rows = y[ci * SI + st * P : ci * SI + (st + 1) * P, :]
            if split:
                # tail latency: copy the two halves on different engines and
                # overlap the two output DMAs
                nc.vector.tensor_copy(ysb[:, 0:256], ps3[:, 0:256])
                nc.scalar.copy(ysb[:, 256:512], ps3[:, 256:512])
                for q in range(2):
                    nc.sync.dma_start(
                        out=rows[:, half * 512 + q * 256 : half * 512 + (q + 1) * 256],
                        in_=ysb[:, q * 256 : (q + 1) * 256],
                    )
            elif eng is nc.scalar:
                nc.scalar.copy(ysb, ps3)
                nc.sync.dma_start(
                    out=rows[:, half * 512 : (half + 1) * 512],
                    in_=ysb,
                )
            else:
                nc.vector.tensor_copy(ysb, ps3)
                nc.sync.dma_start(
                    out=rows[:, half * 512 : (half + 1) * 512],
                    in_=ysb,
                )

        # ---- PE warmup: matmuls on scratch data issued before any DMA so
        # the p-state ramp completes during the startup DMA window --------
        if WARM:
            wsc = pp.tile([P, 256], F16)
            nc.vector.memset(wsc, 0.0)
            wps = ps_u.tile([P, 256], F32, tag="u", name="wps")
            for i in range(WARM):
                nc.tensor.matmul(wps, wsc[:, 0:P], wsc, start=(i == 0), stop=(i == WARM - 1))
        # ---- startup DMAs: interleave chunk-0 x with wq (k-pair batches)
        # so the first projection group's dependencies land earliest ------
        xa0 = xpool.tile([P, NK // 2, SI], F16, tag="xt", name="xa")
        xb0 = xpool.tile([P, NK // 2, SI], F16, tag="xt", name="xb")
        for k2 in range(NK // 4):
            nc.sync.dma_start(out=xa0[:, 2 * k2 : 2 * k2 + 2, :], in_=xt_r[0][:, 2 * k2 : 2 * k2 + 2, :])
            nc.sync.dma_start(out=wq_sb[:, 2 * k2 : 2 * k2 + 2, :], in_=wq_r[:, 2 * k2 : 2 * k2 + 2, :])
        for k2 in range(NK // 4):
            k = NK // 2 + 2 * k2
            nc.sync.dma_start(out=xb0[:, 2 * k2 : 2 * k2 + 2, :], in_=xt_r[0][:, k : k + 2, :])
            nc.sync.dma_start(out=wq_sb[:, k : k + 2, :], in_=wq_r[:, k : k + 2, :])
        nc.sync.dma_start(out=wk_sb[:, 0 : NK // 2, :], in_=wk_r[:, 0 : NK // 2, :])
        nc.sync.dma_start(out=wk_sb[:, NK // 2 : NK, :], in_=wk_r[:, NK // 2 : NK, :])
        nc.sync.dma_start(out=wv_sb[:, 0 : NK // 2, :], in_=wv_r[:, 0 : NK // 2, :])
        nc.sync.dma_start(out=wv_sb[:, NK // 2 : NK, :], in_=wv_r[:, NK // 2 : NK, :])
        nc.sync.dma_start(out=mask_sb, in_=masks[:, :, :])
        nc.sync.dma_start(out=id_sb, in_=ident[:, :])
        nc.sync.dma_start(out=wo_sb, in_=wo.rearrange("(t p) m -> p t m", p=P))

        qt0, groups0 = proj_groups(0, (xa0, xb0))
        for g in groups0:
            g()
        qts = {0: qt0}
        ao_qs = {}
        fillers = deque()
        pre_exs = {}
        pend = None  # (ci, h, exs) - carried ACROSS chunk boundaries so the
        # boundary attn@v is emitted after the next chunk's first scores
        for ci in range(NCI):
            ao_qs[ci] = aoqp.tile([P, 4, HPC, DH], F16, tag="aoq", name="ao_q")
            if ci + 1 < NCI:
                qts[ci + 1], pgroups = proj_groups(ci + 1, emit_x_load(ci + 1))
            else:
                pgroups = []
            pgroups = deque(pgroups)
            qt = qts.pop(ci)
            ngrp = 2 * ci + 2
            npopped = 0
            for h in range(HPC):
                if ci >= 1 and h == 1:
                    for t in range(4):
                        fillers.append(
                            lambda t=t, c=ci - 1: emit_transpose(t, ao_qs[c], aoTs[c])
                        )
                    for st in range(4):
                        for half in range(2):
                            fillers.append(
                                lambda st=st, half=half, c=ci - 1: emit_outproj(c, st, half)
                            )
                exs = pre_exs.pop((ci, h), None)
                if exs is None:
                    exs = emit_scores(ci, h, qt, 0, 2 * ci)
                exs = exs + emit_scores(ci, h, qt, 2 * ci, ngrp)
                if pend is not None:
                    emit_attnv_norm(pend[0], pend[1], pend[2], ao_qs[pend[0]])
                pend = (ci, h, exs)
                for _ in range(int(KPOPS[ci])):
                    if pgroups:
                        pgroups.popleft()()
                        npopped += 1
                if ci + 1 < NCI and ci + 1 >= 1 and KPREH[ci] <= h < KPREH[ci] + PRE:
                    hp = h - KPREH[ci]
                    # the pre-computed scores read qt(ci+1)[:, hp//2, :]; make
                    # sure that projection group has been emitted first
                    while npopped < hp // 2 + 1 and pgroups:
                        pgroups.popleft()()
                        npopped += 1
                    pre_exs[(ci + 1, hp)] = emit_scores(
                        ci + 1, hp, qts[ci + 1], 0, 2 * (ci + 1)
                    )
                npop = FILL[ci]
                if ci == NCI - 1 and len(KF3) == HPC:
                    npop = int(KF3[h])
                for _ in range(npop):
                    if fillers:
                        fillers.popleft()()
            while pgroups:
                pgroups.popleft()()
            for _ in range(FLUSH):
                if fillers:
                    fillers.popleft()()
        emit_attnv_norm(pend[0], pend[1], pend[2], ao_qs[pend[0]])
        for t in range(4):
            emit_transpose(t, ao_qs[NCI - 1], aoTs[NCI - 1],
                           eng=(nc.scalar if t % 2 else nc.vector))
        while fillers:
            fillers.popleft()()
        for st in range(4):
            for half in range(2):
                emit_outproj(NCI - 1, st, half,
                             split=False,
                             eng=(nc.scalar if (2 * st + half) % 2 else nc.vector),
                             pool=(ps_sc if (2 * st + half) % 2 else None))


def build():
    nc = Bacc()
    xt = nc.dram_tensor("xt", [D, S], F16, kind="ExternalInput")
    wq = nc.dram_tensor("wq", [D, AD], F16, kind="ExternalInput")
    wk = nc.dram_tensor("wk", [D, AD], F16, kind="ExternalInput")
    wv = nc.dram_tensor("wv", [D, AD], F16, kind="ExternalInput")
    wo = nc.dram_tensor("wo", [AD, D], F16, kind="ExternalInput")
    masks = nc.dram_tensor("masks", [P, 2, SI], F16, kind="ExternalInput")
    ident = nc.dram_tensor("ident", [P, P], F16, kind="ExternalInput")
    y = nc.dram_tensor("y", [S, D], F32, kind="ExternalOutput")
    with tile.TileContext(nc) as tc:
        _emit(nc, tc, xt, wq, wk, wv, wo, masks, ident, y)
    nc.compile()
    return nc


_NC = None


def _causal_masks():
    p = np.arange(P)[:, None]
    f = np.arange(SI)[None, :]
    return np.stack(
        [(f >= p).astype(np.float32), (f >= p + P).astype(np.float32)], axis=1
    )  # [P, 2, SI]


def run(x, Wq, Wk, Wv, Wo, bo, **run_kwargs):
    global _NC
    x = np.asarray(x, np.float32)
    Wq = np.asarray(Wq, np.float32)
    Wk = np.asarray(Wk, np.float32)
    Wv = np.asarray(Wv, np.float32)
    Wo = np.asarray(Wo, np.float32)
    bo = np.asarray(bo, np.float32)

    if _NC is None:
        _NC = build()

    masks = _causal_masks().astype(np.float16)
    ident = np.eye(P, dtype=np.float16)
    wq_s = Wq * (1.0 / np.sqrt(DH))  # fold the 1/sqrt(dh) score scale into q
    in_maps = []
    for c in range(2 * B):
        b, g = divmod(c, G)
        cols = slice(g * AD, (g + 1) * AD)
        in_maps.append(
            {
                "xt": np.ascontiguousarray(x[b].T).astype(np.float16),
                "wq": np.ascontiguousarray(wq_s[:, cols]).astype(np.float16),
                "wk": np.ascontiguousarray(Wk[:, cols]).astype(np.float16),
                "wv": np.ascontiguousarray(Wv[:, cols]).astype(np.float16),
                "wo": np.ascontiguousarray(Wo[cols, :]).astype(np.float16),
                "masks": masks,
                "ident": ident,
            }
        )

    res = run_bass_kernel_spmd(_NC, in_maps, core_ids=list(range(2 * B)), **run_kwargs)
    ys = [m["y"] for m in res.results]
    out = np.stack([ys[G * b] + ys[G * b + 1] for b in range(B)]) + bo
    return out.astype(np.float32), res


def kernel(**inputs):
    out, _ = run(**inputs)
    return out
